# revision 16
# baseline (speedup 1.0000x reference)
"""Trainium2 Bass kernel for nn_Attention (B=2, N=4096, DIM=768, H=12 heads).

Sharding: 24 (batch, head) pairs over 8 cores -> 3 heads per core, 4 cores
per batch element. Each core computes, for its batch b and its 3 heads:
  q,k,v projections -> full attention (flash-style, no score materialization
  to HBM) -> partial output projection  y_partial^T = sum_h wp_h^T @ O_h^T.
The host sums the 4 partials per batch and adds the bias.

Device dataflow (all matmul inputs bf16, fp32 PSUM accumulation):
  - host passes x^T, w_qk^T, w_v^T, w_p^T pre-transposed/pre-sliced in bf16
  - qk^T = [w_q|w_k]^T.T @ x^T    -> q^T,k^T [64, 4096] per head (d-major)
  - S^T[k,q] = k^T.T @ q^T        -> PSUM, 2-way row-packed (K=64)
  - P^T = exp(S^T * scale)        -> ScalarE (the bottleneck engine)
  - O^T|den = [V|1].T @ P^T       -> PSUM accumulate over k blocks
  - O^T /= den (recip + gpsimd partition-broadcast + DVE mult)
  - y^T += wp_h^T.T @ O_h^T       -> per-head K=64 accumulation
"""

import numpy as np
import ml_dtypes

import concourse.bacc as bacc
import concourse.mybir as mybir
import concourse.tile as tile
from concourse.bass_utils import run_bass_kernel_spmd

BF16 = mybir.dt.bfloat16
F32 = mybir.dt.float32

DIM = 768
N = 4096
NUM_HEADS = 12
HEAD_DIM = 64
SCALE = HEAD_DIM ** -0.5
B = 2
NCORES = 8
HPC = 3  # heads per core
CCH = DIM // 128  # 6 contraction chunks of 128
NQT = 8  # q tiles of 512
QT = 512
NKB = 32  # k blocks of 128
KB = 128
GRP = 2  # k-blocks per exp group (2 PSUM banks, aligns with row-pack pairs)


def build_program():
    nc = bacc.Bacc("TRN2", target_bir_lowering=False, debug=False)

    xT = nc.dram_tensor("xT", [DIM, N], BF16, kind="ExternalInput")
    wqkT = nc.dram_tensor("wqkT", [DIM, HPC * 128], BF16, kind="ExternalInput")
    wvT = nc.dram_tensor("wvT", [DIM, HPC * 64], BF16, kind="ExternalInput")
    wpT = nc.dram_tensor("wpT", [HPC * 64, DIM], BF16, kind="ExternalInput")
    yT = nc.dram_tensor("yT", [DIM, N], F32, kind="ExternalOutput")

    ngrp = NKB // GRP

    with tile.TileContext(nc) as tc:
        with (
            tc.tile_pool(name="wpool", bufs=1) as wpool,
            tc.tile_pool(name="qkpool", bufs=1) as qkpool,
            tc.tile_pool(name="pspool", bufs=2, space="PSUM") as pspool,
            tc.tile_pool(name="vpool", bufs=1, space="PSUM") as vpool,
            tc.tile_pool(name="accpool", bufs=1, space="PSUM") as accpool,
            tc.tile_pool(name="espool", bufs=18) as espool,
            tc.tile_pool(name="dpool", bufs=3) as dpool,
            tc.tile_pool(name="opool", bufs=19) as opool,
            tc.tile_pool(name="ypool", bufs=4) as ypool,
        ):
            # xT split into one tile per q-token-tile so phase A can start as
            # soon as the first slice lands (per-tile DMA dependencies).
            xTs = [
                wpool.tile([128, CCH * QT], BF16, tag=f"xT{qt}", name=f"xT{qt}")
                for qt in range(NQT)
            ]
            wqk_sb = wpool.tile([128, CCH * HPC * 128], BF16, tag="wqk")
            wv_sb = wpool.tile([128, CCH * HPC * 64], BF16, tag="wv")
            wp_sb = wpool.tile([64, HPC * DIM], BF16, tag="wp")
            T = [
                qkpool.tile([128, 2 * N], BF16, tag=f"T{h}", name=f"T{h}")
                for h in range(HPC)
            ]
            V = [
                qkpool.tile([128, NKB * 65], BF16, tag=f"V{h}", name=f"V{h}")
                for h in range(HPC)
            ]

            # consolidated multi-dim DMAs: one instruction per destination
            # tile keeps the Sync sequencer FIFO short (it issues serially).
            wqk_src = wqkT[:].rearrange("(c p) n -> p c n", p=128)
            nc.sync.dma_start(
                out=wqk_sb[:].rearrange("p (c n) -> p c n", n=384), in_=wqk_src
            )
            xT_src = xT[:].rearrange("(c p) n -> p c n", p=128)
            for qt in range(NQT):
                nc.sync.dma_start(
                    out=xTs[qt][:].rearrange("p (c n) -> p c n", n=QT),
                    in_=xT_src[:, :, qt * QT:(qt + 1) * QT],
                )
            nc.sync.dma_start(
                out=wv_sb[:].rearrange("p (c n) -> p c n", n=192),
                in_=wvT[:].rearrange("(c p) n -> p c n", p=128),
            )
            nc.sync.dma_start(
                out=wp_sb[0:64, :].rearrange("p (h n) -> p h n", n=DIM),
                in_=wpT[:].rearrange("(h p) n -> p h n", p=64),
            )
            for h in range(HPC):
                nc.gpsimd.memset(V[h][:], 1.0)

            # HAM warmup: keep the PE busy during the input DMA wait so the
            # clock gate is at 8/8 when the real matmuls start (~3.4us ramp).
            warm = accpool.tile([128, 448], F32, tag="yb", name="warm")
            for i in range(34):
                nc.tensor.matmul(
                    warm[:],
                    lhsT=wqk_sb[:, 0:128],
                    rhs=wqk_sb[:, 0:448],
                    start=True,
                    stop=True,
                )

            # ---- emission helpers ----
            def emit_qk_tile(h, qt):
                ps = pspool.tile([128, QT], F32, tag="s", name=f"qk{h}_{qt}")
                for c in range(CCH):
                    nc.tensor.matmul(
                        ps[:],
                        lhsT=wqk_sb[:, c * 384 + h * 128: c * 384 + (h + 1) * 128],
                        rhs=xTs[qt][:, c * QT:(c + 1) * QT],
                        start=(c == 0),
                        stop=(c == CCH - 1),
                    )
                nc.vector.tensor_copy(
                    T[h][0:64, N + qt * QT: N + (qt + 1) * QT], ps[0:64, :]
                )
                nc.vector.tensor_copy(
                    T[h][64:128, qt * QT:(qt + 1) * QT], ps[64:128, :]
                )

            def emit_qk_dup(h):
                nc.sync.dma_start(out=T[h][0:64, 0:N], in_=T[h][64:128, 0:N])
                nc.sync.dma_start(out=T[h][64:128, N:2 * N], in_=T[h][0:64, N:2 * N])

            def emit_v_tile(tt):
                ps = vpool.tile([128, HPC * 64], F32, tag="v", name=f"v{tt}")
                for c in range(CCH):
                    nc.tensor.matmul(
                        ps[:],
                        lhsT=xTs[tt // 4][:, c * QT + (tt % 4) * 128: c * QT + (tt % 4) * 128 + 128],
                        rhs=wv_sb[:, c * 192:(c + 1) * 192],
                        start=(c == 0),
                        stop=(c == CCH - 1),
                    )
                for h in range(HPC):
                    nc.vector.tensor_copy(
                        V[h][:, tt * 65: tt * 65 + 64],
                        ps[:, h * 64:(h + 1) * 64],
                    )

            es_store = {}

            def emit_s_group(h, qt, g):
                ps = pspool.tile([128, GRP * QT], F32, tag="s", name=f"ps{h}_{qt}_{g}")
                es = espool.tile([128, GRP * QT], BF16, tag="es", name=f"es{h}_{qt}_{g}")
                for j in range(GRP):
                    kb = g * GRP + j
                    o = 64 * (kb % 2)
                    nc.tensor.matmul(
                        ps[:, j * QT:(j + 1) * QT],
                        lhsT=T[h][o:o + 64, kb * KB:(kb + 1) * KB],
                        rhs=T[h][o:o + 64, N + qt * QT: N + (qt + 1) * QT],
                        start=True,
                        stop=True,
                    )
                nc.scalar.activation(
                    es[:], ps[:], mybir.ActivationFunctionType.Exp, scale=SCALE
                )
                es_store[(h, qt, g)] = es

            def emit_av_group(h, qt, g, po):
                es = es_store.pop((h, qt, g))
                for j in range(GRP):
                    kb = g * GRP + j
                    nc.tensor.matmul(
                        po[:],
                        lhsT=V[h][:, kb * 65: kb * 65 + 65],
                        rhs=es[:, j * QT:(j + 1) * QT],
                        start=(kb == 0),
                        stop=(kb == NKB - 1),
                        skip_group_check=True,
                    )

            O = [[None] * HPC for _ in range(NQT)]

            def emit_norm(h, qt, po):
                # reciprocal_approx_fast misreads PSUM sources on HW — bounce
                # the denominator row through SBUF first.
                dr0 = dpool.tile([1, QT], F32, tag="dr0", name=f"dr0_{h}_{qt}")
                nc.vector.tensor_copy(dr0[:], po[64:65, :])
                dr = dpool.tile([1, QT], F32, tag="dr", name=f"dr{h}_{qt}")
                nc.vector.reciprocal_approx_fast(out=dr[:], in_=dr0[:])
                db = dpool.tile([64, QT], F32, tag="db", name=f"db{h}_{qt}")
                nc.gpsimd.partition_broadcast(db[:], dr[:])
                oh = opool.tile([64, QT], BF16, tag="O", name=f"O{h}_{qt}")
                nc.vector.tensor_mul(oh[:], po[0:64, :], db[:])
                O[qt][h] = oh

            def emit_attn(h, qt, filler=None):
                """One (head, q-tile): an uninterrupted S-pair run (keeps the
                PE in 64-row mode with LDW hidden by parity alternation), then
                the AV block; buffer pressure paces both against ACT."""
                po = accpool.tile([65, QT], F32, tag="o", bufs=2, name=f"po{h}_{qt}")
                for g in range(ngrp):
                    emit_s_group(h, qt, g)
                if filler is not None:
                    filler()
                for g in range(ngrp):
                    emit_av_group(h, qt, g, po)
                emit_norm(h, qt, po)

            def emit_proj(qt):
                for oc in range(CCH):
                    py = accpool.tile([128, QT], F32, tag="yb", name=f"py{qt}_{oc}")
                    for h in range(HPC):
                        nc.tensor.matmul(
                            py[:],
                            lhsT=wp_sb[0:64, h * DIM + oc * 128: h * DIM + (oc + 1) * 128],
                            rhs=O[qt][h][:],
                            start=(h == 0),
                            stop=(h == HPC - 1),
                        )
                    ysb = ypool.tile([128, QT], F32, tag="y", name=f"y{qt}_{oc}")
                    nc.vector.tensor_copy(ysb[:], py[:])
                    nc.sync.dma_start(
                        out=yT[oc * 128:(oc + 1) * 128, qt * QT:(qt + 1) * QT],
                        in_=ysb[:],
                    )

            # ---- static schedule ----
            for qt in range(NQT):
                emit_qk_tile(0, qt)
            emit_qk_dup(0)

            # head 0, q-tile 0: S+exp first (feeds ACT early), v while ACT
            # chews, then the deferred AV for qt0.
            po00 = accpool.tile([65, QT], F32, tag="o", bufs=2, name="po0_0")
            for g in range(ngrp):
                emit_s_group(0, 0, g)
            for tt in range(NKB):
                emit_v_tile(tt)
            for g in range(ngrp):
                emit_av_group(0, 0, g, po00)
            emit_norm(0, 0, po00)

            def qk_filler(nexth, qt):
                if qt < NQT - 1:
                    return lambda: emit_qk_tile(nexth, qt - 1)

                def last():
                    emit_qk_tile(nexth, NQT - 2)
                    emit_qk_tile(nexth, NQT - 1)
                    emit_qk_dup(nexth)

                return last

            for qt in range(1, NQT):
                emit_attn(0, qt, filler=qk_filler(1, qt))

            for qt in range(NQT):
                filler = qk_filler(2, qt) if qt >= 1 else None
                emit_attn(1, qt, filler=filler)

            for qt in range(NQT):
                filler = (lambda q=qt: emit_proj(q - 1)) if qt >= 1 else None
                emit_attn(2, qt, filler=filler)
            emit_proj(NQT - 1)

    nc.compile()
    return nc


def make_in_maps(x, w_qkv):
    """Build the 8 per-core input maps from the full fp32 inputs."""
    bf = ml_dtypes.bfloat16
    in_maps = []
    for core in range(NCORES):
        b = core // 4
        hs = [(core % 4) * HPC + i for i in range(HPC)]
        xTb = np.ascontiguousarray(np.asarray(x[b]).T).astype(bf)
        wqk = np.empty((DIM, HPC * 128), dtype=bf)
        wv = np.empty((DIM, HPC * 64), dtype=bf)
        for i, h in enumerate(hs):
            wqk[:, i * 128: i * 128 + 64] = w_qkv[h * 64:(h + 1) * 64, :].T
            wqk[:, i * 128 + 64: i * 128 + 128] = w_qkv[DIM + h * 64: DIM + (h + 1) * 64, :].T
            wv[:, i * 64:(i + 1) * 64] = w_qkv[2 * DIM + h * 64: 2 * DIM + (h + 1) * 64, :].T
        in_maps.append({"xT": xTb, "wqkT": wqk, "wvT": wv})
    return in_maps


def make_wp_map(core, w_proj):
    bf = ml_dtypes.bfloat16
    hs = [(core % 4) * HPC + i for i in range(HPC)]
    wp = np.empty((HPC * 64, DIM), dtype=bf)
    for i, h in enumerate(hs):
        wp[i * 64:(i + 1) * 64, :] = w_proj[:, h * 64:(h + 1) * 64].T
    return wp


_NC = None


def kernel(x, w_qkv, w_proj, b_proj):
    global _NC
    if _NC is None:
        _NC = build_program()
    x = np.asarray(x, dtype=np.float32)
    w_qkv = np.asarray(w_qkv, dtype=np.float32)
    w_proj = np.asarray(w_proj, dtype=np.float32)
    b_proj = np.asarray(b_proj, dtype=np.float32)

    in_maps = make_in_maps(x, w_qkv)
    for core in range(NCORES):
        in_maps[core]["wpT"] = make_wp_map(core, w_proj)

    r = run_bass_kernel_spmd(_NC, in_maps, list(range(NCORES)))
    y = np.zeros((B, N, DIM), dtype=np.float32)
    for core in range(NCORES):
        b = core // 4
        y[b] += r.results[core]["yT"].T
    y += b_proj[None, None, :]
    return y


# revision 24
# speedup vs baseline: 1.0452x; 1.0452x over previous
"""Trainium2 Bass kernel for nn_Attention (B=2, N=4096, DIM=768, H=12 heads).

Sharding: 24 (batch, head) pairs over 8 cores -> 3 heads per core, 4 cores
per batch element. Each core computes, for its batch b and its 3 heads:
  q,k,v projections -> full attention (flash-style, no score materialization
  to HBM) -> partial output projection  y_partial^T = sum_h wp_h^T @ O_h^T.
The host sums the 4 partials per batch and adds the bias.

Device dataflow (all matmul inputs bf16, fp32 PSUM accumulation):
  - host passes x^T, w_qk^T, w_v^T, w_p^T pre-transposed/pre-sliced in bf16
  - qk^T = [w_q|w_k]^T.T @ x^T    -> q^T,k^T [64, 4096] per head (d-major)
  - S^T[k,q] = k^T.T @ q^T        -> PSUM, 2-way row-packed (K=64)
  - P^T = exp(S^T * scale)        -> ScalarE (the bottleneck engine)
  - O^T|den = [V|1].T @ P^T       -> PSUM accumulate over k blocks
  - O^T /= den (recip + gpsimd partition-broadcast + DVE mult)
  - y^T += wp_h^T.T @ O_h^T       -> per-head K=64 accumulation
"""

import numpy as np
import ml_dtypes

import concourse.bacc as bacc
import concourse.mybir as mybir
import concourse.tile as tile
from concourse.bass_utils import run_bass_kernel_spmd

BF16 = mybir.dt.bfloat16
F32 = mybir.dt.float32

DIM = 768
N = 4096
NUM_HEADS = 12
HEAD_DIM = 64
SCALE = HEAD_DIM ** -0.5
B = 2
NCORES = 8
HPC = 3  # heads per core
CCH = DIM // 128  # 6 contraction chunks of 128
NQT = 8  # q tiles of 512
QT = 512
NKB = 32  # k blocks of 128
KB = 128
GRP = 2  # k-blocks per exp group (2 PSUM banks, aligns with row-pack pairs)


def build_program():
    nc = bacc.Bacc("TRN2", target_bir_lowering=False, debug=False)

    xT = nc.dram_tensor("xT", [DIM, N], BF16, kind="ExternalInput")
    wqkT = nc.dram_tensor("wqkT", [DIM, HPC * 128], BF16, kind="ExternalInput")
    wvT = nc.dram_tensor("wvT", [DIM, HPC * 64], BF16, kind="ExternalInput")
    wpT = nc.dram_tensor("wpT", [HPC * 64, DIM], BF16, kind="ExternalInput")
    yT = nc.dram_tensor("yT", [DIM, N], F32, kind="ExternalOutput")

    ngrp = NKB // GRP

    with tile.TileContext(nc) as tc:
        with (
            tc.tile_pool(name="wpool", bufs=1) as wpool,
            tc.tile_pool(name="qkpool", bufs=1) as qkpool,
            tc.tile_pool(name="pspool", bufs=2, space="PSUM") as pspool,
            tc.tile_pool(name="vpool", bufs=1, space="PSUM") as vpool,
            tc.tile_pool(name="accpool", bufs=1, space="PSUM") as accpool,
            tc.tile_pool(name="espool", bufs=18) as espool,
            tc.tile_pool(name="dpool", bufs=3) as dpool,
            tc.tile_pool(name="opool", bufs=19) as opool,
            tc.tile_pool(name="ypool", bufs=4) as ypool,
        ):
            # xT split into one tile per q-token-tile so phase A can start as
            # soon as the first slice lands (per-tile DMA dependencies).
            xTs = [
                wpool.tile([128, CCH * QT], BF16, tag=f"xT{qt}", name=f"xT{qt}")
                for qt in range(NQT)
            ]
            wqk_sb = wpool.tile([128, CCH * HPC * 128], BF16, tag="wqk")
            wv_sb = wpool.tile([128, CCH * HPC * 64], BF16, tag="wv")
            wp_sb = wpool.tile([64, HPC * DIM], BF16, tag="wp")
            T = [
                qkpool.tile([128, 2 * N], BF16, tag=f"T{h}", name=f"T{h}")
                for h in range(HPC)
            ]
            V = [
                qkpool.tile([128, NKB * 65], BF16, tag=f"V{h}", name=f"V{h}")
                for h in range(HPC)
            ]

            # consolidated multi-dim DMAs: one instruction per destination
            # tile keeps the Sync sequencer FIFO short (it issues serially).
            wqk_src = wqkT[:].rearrange("(c p) n -> p c n", p=128)
            nc.sync.dma_start(
                out=wqk_sb[:].rearrange("p (c n) -> p c n", n=384), in_=wqk_src
            )
            xT_src = xT[:].rearrange("(c p) n -> p c n", p=128)
            xT_loaded = set()

            def load_xT(qt):
                if qt in xT_loaded:
                    return
                xT_loaded.add(qt)
                nc.sync.dma_start(
                    out=xTs[qt][:].rearrange("p (c n) -> p c n", n=QT),
                    in_=xT_src[:, :, qt * QT:(qt + 1) * QT],
                )
            def load_w2():
                nc.sync.dma_start(
                    out=wv_sb[:].rearrange("p (c n) -> p c n", n=192),
                    in_=wvT[:].rearrange("(c p) n -> p c n", p=128),
                )
                nc.sync.dma_start(
                    out=wp_sb[0:64, :].rearrange("p (h n) -> p h n", n=DIM),
                    in_=wpT[:].rearrange("(h p) n -> p h n", p=64),
                )
            for h in range(HPC):
                nc.gpsimd.memset(V[h][:], 1.0)

            # ---- emission helpers ----
            def emit_qk_tile(h, qt):
                load_xT(qt)
                ps = pspool.tile([128, QT], F32, tag="s", name=f"qk{h}_{qt}")
                for c in range(CCH):
                    nc.tensor.matmul(
                        ps[:],
                        lhsT=wqk_sb[:, c * 384 + h * 128: c * 384 + (h + 1) * 128],
                        rhs=xTs[qt][:, c * QT:(c + 1) * QT],
                        start=(c == 0),
                        stop=(c == CCH - 1),
                    )
                nc.vector.tensor_copy(
                    T[h][0:64, N + qt * QT: N + (qt + 1) * QT], ps[0:64, :]
                )
                nc.vector.tensor_copy(
                    T[h][64:128, qt * QT:(qt + 1) * QT], ps[64:128, :]
                )

            def emit_qk_dup(h, qt=None):
                if qt is None:
                    nc.sync.dma_start(out=T[h][0:64, 0:N], in_=T[h][64:128, 0:N])
                    nc.sync.dma_start(
                        out=T[h][64:128, N:2 * N], in_=T[h][0:64, N:2 * N]
                    )
                else:
                    s = qt * QT
                    nc.sync.dma_start(
                        out=T[h][0:64, s:s + QT], in_=T[h][64:128, s:s + QT]
                    )
                    nc.sync.dma_start(
                        out=T[h][64:128, N + s: N + s + QT],
                        in_=T[h][0:64, N + s: N + s + QT],
                    )

            def emit_v_tile(tt):
                ps = vpool.tile([128, HPC * 64], F32, tag="v", name=f"v{tt}")
                for c in range(CCH):
                    nc.tensor.matmul(
                        ps[:],
                        lhsT=xTs[tt // 4][:, c * QT + (tt % 4) * 128: c * QT + (tt % 4) * 128 + 128],
                        rhs=wv_sb[:, c * 192:(c + 1) * 192],
                        start=(c == 0),
                        stop=(c == CCH - 1),
                    )
                for h in range(HPC):
                    nc.vector.tensor_copy(
                        V[h][:, tt * 65: tt * 65 + 64],
                        ps[:, h * 64:(h + 1) * 64],
                    )

            es_store = {}

            def emit_s_group(h, qt, g):
                ps = pspool.tile([128, GRP * QT], F32, tag="s", name=f"ps{h}_{qt}_{g}")
                es = espool.tile([128, GRP * QT], BF16, tag="es", name=f"es{h}_{qt}_{g}")
                for j in range(GRP):
                    kb = g * GRP + j
                    o = 64 * (kb % 2)
                    nc.tensor.matmul(
                        ps[:, j * QT:(j + 1) * QT],
                        lhsT=T[h][o:o + 64, kb * KB:(kb + 1) * KB],
                        rhs=T[h][o:o + 64, N + qt * QT: N + (qt + 1) * QT],
                        start=True,
                        stop=True,
                    )
                nc.scalar.activation(
                    es[:], ps[:], mybir.ActivationFunctionType.Exp, scale=SCALE
                )
                es_store[(h, qt, g)] = es

            def emit_av_group(h, qt, g, po):
                es = es_store.pop((h, qt, g))
                for j in range(GRP):
                    kb = g * GRP + j
                    nc.tensor.matmul(
                        po[:],
                        lhsT=V[h][:, kb * 65: kb * 65 + 65],
                        rhs=es[:, j * QT:(j + 1) * QT],
                        start=(kb == 0),
                        stop=(kb == NKB - 1),
                        skip_group_check=True,
                    )

            O = [[None] * HPC for _ in range(NQT)]

            def emit_norm(h, qt, po):
                # reciprocal_approx_fast misreads PSUM sources on HW — bounce
                # the denominator row through SBUF first.
                dr0 = dpool.tile([1, QT], F32, tag="dr0", name=f"dr0_{h}_{qt}")
                nc.vector.tensor_copy(dr0[:], po[64:65, :])
                dr = dpool.tile([1, QT], F32, tag="dr", name=f"dr{h}_{qt}")
                nc.vector.reciprocal_approx_fast(out=dr[:], in_=dr0[:])
                db = dpool.tile([64, QT], F32, tag="db", name=f"db{h}_{qt}")
                nc.gpsimd.partition_broadcast(db[:], dr[:])
                oh = opool.tile([64, QT], BF16, tag="O", name=f"O{h}_{qt}")
                nc.vector.tensor_mul(oh[:], po[0:64, :], db[:])
                O[qt][h] = oh

            def emit_attn(h, qt, filler=None, interleave=False):
                """One (head, q-tile): an uninterrupted S-pair run (keeps the
                PE in 64-row mode with LDW hidden by parity alternation), then
                the AV block; buffer pressure paces both against ACT. The
                final tile interleaves AV so the kernel tail is short."""
                po = accpool.tile([65, QT], F32, tag="o", bufs=2, name=f"po{h}_{qt}")
                if interleave:
                    for g in range(ngrp):
                        emit_s_group(h, qt, g)
                        if g == 1 and filler is not None:
                            filler()
                        if g >= 2:
                            emit_av_group(h, qt, g - 2, po)
                    emit_av_group(h, qt, ngrp - 2, po)
                    emit_av_group(h, qt, ngrp - 1, po)
                    emit_norm(h, qt, po)
                    return
                for g in range(ngrp):
                    emit_s_group(h, qt, g)
                if filler is not None:
                    filler()
                for g in range(ngrp):
                    emit_av_group(h, qt, g, po)
                emit_norm(h, qt, po)

            def emit_proj(qt):
                for oc in range(CCH):
                    py = accpool.tile([128, QT], F32, tag="yb", name=f"py{qt}_{oc}")
                    for h in range(HPC):
                        nc.tensor.matmul(
                            py[:],
                            lhsT=wp_sb[0:64, h * DIM + oc * 128: h * DIM + (oc + 1) * 128],
                            rhs=O[qt][h][:],
                            start=(h == 0),
                            stop=(h == HPC - 1),
                        )
                    ysb = ypool.tile([128, QT], F32, tag="y", name=f"y{qt}_{oc}")
                    nc.vector.tensor_copy(ysb[:], py[:])
                    nc.sync.dma_start(
                        out=yT[oc * 128:(oc + 1) * 128, qt * QT:(qt + 1) * QT],
                        in_=ysb[:],
                    )

            # ---- static schedule ----
            # head 0 qk tiles emitted incrementally with qt0's S groups: the
            # first exp fires after just two qk tiles instead of all eight.
            emit_qk_tile(0, 0)
            emit_qk_dup(0, 0)
            emit_qk_tile(0, 1)
            emit_qk_dup(0, 1)
            load_w2()
            for j in range(2, NQT):
                emit_s_group(0, 0, 2 * (j - 2))
                emit_s_group(0, 0, 2 * (j - 2) + 1)
                emit_qk_tile(0, j)
                emit_qk_dup(0, j)
            for g in range(12, ngrp):
                emit_s_group(0, 0, g)

            # v while ACT chews on qt0's exps, then the deferred AV for qt0.
            po00 = accpool.tile([65, QT], F32, tag="o", bufs=2, name="po0_0")
            for tt in range(NKB):
                emit_v_tile(tt)
            for g in range(ngrp):
                emit_av_group(0, 0, g, po00)
            emit_norm(0, 0, po00)

            def qk_filler(nexth, qt):
                if qt < NQT - 1:
                    return lambda: emit_qk_tile(nexth, qt - 1)

                def last():
                    emit_qk_tile(nexth, NQT - 2)
                    emit_qk_tile(nexth, NQT - 1)
                    emit_qk_dup(nexth)

                return last

            for qt in range(1, NQT):
                emit_attn(0, qt, filler=qk_filler(1, qt))

            for qt in range(NQT):
                filler = qk_filler(2, qt) if qt >= 1 else None
                emit_attn(1, qt, filler=filler)

            for qt in range(NQT):
                filler = (lambda q=qt: emit_proj(q - 1)) if qt >= 1 else None
                emit_attn(2, qt, filler=filler, interleave=(qt == NQT - 1))
            emit_proj(NQT - 1)

    nc.compile()
    return nc


def make_in_maps(x, w_qkv):
    """Build the 8 per-core input maps from the full fp32 inputs."""
    bf = ml_dtypes.bfloat16
    in_maps = []
    for core in range(NCORES):
        b = core // 4
        hs = [(core % 4) * HPC + i for i in range(HPC)]
        xTb = np.ascontiguousarray(np.asarray(x[b]).T).astype(bf)
        wqk = np.empty((DIM, HPC * 128), dtype=bf)
        wv = np.empty((DIM, HPC * 64), dtype=bf)
        for i, h in enumerate(hs):
            wqk[:, i * 128: i * 128 + 64] = w_qkv[h * 64:(h + 1) * 64, :].T
            wqk[:, i * 128 + 64: i * 128 + 128] = w_qkv[DIM + h * 64: DIM + (h + 1) * 64, :].T
            wv[:, i * 64:(i + 1) * 64] = w_qkv[2 * DIM + h * 64: 2 * DIM + (h + 1) * 64, :].T
        in_maps.append({"xT": xTb, "wqkT": wqk, "wvT": wv})
    return in_maps


def make_wp_map(core, w_proj):
    bf = ml_dtypes.bfloat16
    hs = [(core % 4) * HPC + i for i in range(HPC)]
    wp = np.empty((HPC * 64, DIM), dtype=bf)
    for i, h in enumerate(hs):
        wp[i * 64:(i + 1) * 64, :] = w_proj[:, h * 64:(h + 1) * 64].T
    return wp


_NC = None


def kernel(x, w_qkv, w_proj, b_proj):
    global _NC
    if _NC is None:
        _NC = build_program()
    x = np.asarray(x, dtype=np.float32)
    w_qkv = np.asarray(w_qkv, dtype=np.float32)
    w_proj = np.asarray(w_proj, dtype=np.float32)
    b_proj = np.asarray(b_proj, dtype=np.float32)

    in_maps = make_in_maps(x, w_qkv)
    for core in range(NCORES):
        in_maps[core]["wpT"] = make_wp_map(core, w_proj)

    r = run_bass_kernel_spmd(_NC, in_maps, list(range(NCORES)))
    y = np.zeros((B, N, DIM), dtype=np.float32)
    for core in range(NCORES):
        b = core // 4
        y[b] += r.results[core]["yT"].T
    y += b_proj[None, None, :]
    return y


# revision 25
# speedup vs baseline: 1.0702x; 1.0240x over previous
"""Trainium2 Bass kernel for nn_Attention (B=2, N=4096, DIM=768, H=12 heads).

Sharding: 24 (batch, head) pairs over 8 cores -> 3 heads per core, 4 cores
per batch element. Each core computes, for its batch b and its 3 heads:
  q,k,v projections -> full attention (flash-style, no score materialization
  to HBM) -> partial output projection  y_partial^T = sum_h wp_h^T @ O_h^T.
The host sums the 4 partials per batch and adds the bias.

Device dataflow (all matmul inputs bf16, fp32 PSUM accumulation):
  - host passes x^T, w_qk^T, w_v^T, w_p^T pre-transposed/pre-sliced in bf16
  - qk^T = [w_q|w_k]^T.T @ x^T    -> q^T,k^T [64, 4096] per head (d-major)
  - S^T[k,q] = k^T.T @ q^T        -> PSUM, 2-way row-packed (K=64)
  - P^T = exp(S^T * scale)        -> ScalarE (the bottleneck engine)
  - O^T|den = [V|1].T @ P^T       -> PSUM accumulate over k blocks
  - O^T /= den (recip + gpsimd partition-broadcast + DVE mult)
  - y^T += wp_h^T.T @ O_h^T       -> per-head K=64 accumulation
"""

import numpy as np
import ml_dtypes

import concourse.bacc as bacc
import concourse.mybir as mybir
import concourse.tile as tile
from concourse.bass_utils import run_bass_kernel_spmd

BF16 = mybir.dt.bfloat16
F32 = mybir.dt.float32

DIM = 768
N = 4096
NUM_HEADS = 12
HEAD_DIM = 64
SCALE = HEAD_DIM ** -0.5
B = 2
NCORES = 8
HPC = 3  # heads per core
CCH = DIM // 128  # 6 contraction chunks of 128
NQT = 8  # q tiles of 512
QT = 512
NKB = 32  # k blocks of 128
KB = 128
GRP = 2  # k-blocks per exp group (2 PSUM banks, aligns with row-pack pairs)


def build_program():
    nc = bacc.Bacc("TRN2", target_bir_lowering=False, debug=False)

    xT = nc.dram_tensor("xT", [DIM, N], BF16, kind="ExternalInput")
    wqkT = nc.dram_tensor("wqkT", [DIM, HPC * 128], BF16, kind="ExternalInput")
    wvT = nc.dram_tensor("wvT", [DIM, HPC * 64], BF16, kind="ExternalInput")
    wpT = nc.dram_tensor("wpT", [HPC * 64, DIM], BF16, kind="ExternalInput")
    yT = nc.dram_tensor("yT", [DIM, N], F32, kind="ExternalOutput")

    ngrp = NKB // GRP

    with tile.TileContext(nc) as tc:
        with (
            tc.tile_pool(name="wpool", bufs=1) as wpool,
            tc.tile_pool(name="qkpool", bufs=1) as qkpool,
            tc.tile_pool(name="pspool", bufs=2, space="PSUM") as pspool,
            tc.tile_pool(name="vpool", bufs=1, space="PSUM") as vpool,
            tc.tile_pool(name="accpool", bufs=1, space="PSUM") as accpool,
            tc.tile_pool(name="espool", bufs=18) as espool,
            tc.tile_pool(name="dpool", bufs=3) as dpool,
            tc.tile_pool(name="opool", bufs=19) as opool,
            tc.tile_pool(name="ypool", bufs=4) as ypool,
        ):
            # xT split into one tile per q-token-tile so phase A can start as
            # soon as the first slice lands (per-tile DMA dependencies).
            xTs = [
                wpool.tile([128, CCH * QT], BF16, tag=f"xT{qt}", name=f"xT{qt}")
                for qt in range(NQT)
            ]
            wqk_sb = wpool.tile([128, CCH * HPC * 128], BF16, tag="wqk")
            wv_sb = wpool.tile([128, CCH * HPC * 64], BF16, tag="wv")
            wp_sb = wpool.tile([64, HPC * DIM], BF16, tag="wp")
            T = [
                qkpool.tile([128, 2 * N], BF16, tag=f"T{h}", name=f"T{h}")
                for h in range(HPC)
            ]
            V = [
                qkpool.tile([128, NKB * 65], BF16, tag=f"V{h}", name=f"V{h}")
                for h in range(HPC)
            ]

            # consolidated multi-dim DMAs: one instruction per destination
            # tile keeps the Sync sequencer FIFO short (it issues serially).
            wqk_src = wqkT[:].rearrange("(c p) n -> p c n", p=128)
            nc.sync.dma_start(
                out=wqk_sb[:].rearrange("p (c n) -> p c n", n=384), in_=wqk_src
            )
            xT_src = xT[:].rearrange("(c p) n -> p c n", p=128)
            xT_loaded = set()

            def load_xT(qt):
                if qt in xT_loaded:
                    return
                xT_loaded.add(qt)
                nc.sync.dma_start(
                    out=xTs[qt][:].rearrange("p (c n) -> p c n", n=QT),
                    in_=xT_src[:, :, qt * QT:(qt + 1) * QT],
                )
            def load_w2():
                nc.sync.dma_start(
                    out=wv_sb[:].rearrange("p (c n) -> p c n", n=192),
                    in_=wvT[:].rearrange("(c p) n -> p c n", p=128),
                )
                nc.sync.dma_start(
                    out=wp_sb[0:64, :].rearrange("p (h n) -> p h n", n=DIM),
                    in_=wpT[:].rearrange("(h p) n -> p h n", p=64),
                )
            for h in range(HPC):
                nc.gpsimd.memset(V[h][:], 1.0)

            # ---- emission helpers ----
            def emit_qk_tile(h, qt):
                load_xT(qt)
                ps = pspool.tile([128, QT], F32, tag="s", name=f"qk{h}_{qt}")
                for c in range(CCH):
                    nc.tensor.matmul(
                        ps[:],
                        lhsT=wqk_sb[:, c * 384 + h * 128: c * 384 + (h + 1) * 128],
                        rhs=xTs[qt][:, c * QT:(c + 1) * QT],
                        start=(c == 0),
                        stop=(c == CCH - 1),
                    )
                nc.vector.tensor_copy(
                    T[h][0:64, N + qt * QT: N + (qt + 1) * QT], ps[0:64, :]
                )
                nc.vector.tensor_copy(
                    T[h][64:128, qt * QT:(qt + 1) * QT], ps[64:128, :]
                )

            def emit_qk_dup(h, qt=None):
                if qt is None:
                    nc.sync.dma_start(out=T[h][0:64, 0:N], in_=T[h][64:128, 0:N])
                    nc.sync.dma_start(
                        out=T[h][64:128, N:2 * N], in_=T[h][0:64, N:2 * N]
                    )
                else:
                    s = qt * QT
                    nc.sync.dma_start(
                        out=T[h][0:64, s:s + QT], in_=T[h][64:128, s:s + QT]
                    )
                    nc.sync.dma_start(
                        out=T[h][64:128, N + s: N + s + QT],
                        in_=T[h][0:64, N + s: N + s + QT],
                    )

            def emit_v_tile(tt):
                ps = vpool.tile([128, HPC * 64], F32, tag="v", name=f"v{tt}")
                for c in range(CCH):
                    nc.tensor.matmul(
                        ps[:],
                        lhsT=xTs[tt // 4][:, c * QT + (tt % 4) * 128: c * QT + (tt % 4) * 128 + 128],
                        rhs=wv_sb[:, c * 192:(c + 1) * 192],
                        start=(c == 0),
                        stop=(c == CCH - 1),
                    )
                for h in range(HPC):
                    nc.vector.tensor_copy(
                        V[h][:, tt * 65: tt * 65 + 64],
                        ps[:, h * 64:(h + 1) * 64],
                    )

            es_store = {}

            def emit_s_group(h, qt, g):
                ps = pspool.tile([128, GRP * QT], F32, tag="s", name=f"ps{h}_{qt}_{g}")
                es = espool.tile([128, GRP * QT], BF16, tag="es", name=f"es{h}_{qt}_{g}")
                for j in range(GRP):
                    kb = g * GRP + j
                    o = 64 * (kb % 2)
                    nc.tensor.matmul(
                        ps[:, j * QT:(j + 1) * QT],
                        lhsT=T[h][o:o + 64, kb * KB:(kb + 1) * KB],
                        rhs=T[h][o:o + 64, N + qt * QT: N + (qt + 1) * QT],
                        start=True,
                        stop=True,
                    )
                nc.scalar.activation(
                    es[:], ps[:], mybir.ActivationFunctionType.Exp, scale=SCALE
                )
                es_store[(h, qt, g)] = es

            def emit_av_group(h, qt, g, po):
                es = es_store.pop((h, qt, g))
                for j in range(GRP):
                    kb = g * GRP + j
                    nc.tensor.matmul(
                        po[:],
                        lhsT=V[h][:, kb * 65: kb * 65 + 65],
                        rhs=es[:, j * QT:(j + 1) * QT],
                        start=(kb == 0),
                        stop=(kb == NKB - 1),
                        skip_group_check=True,
                    )

            O = [[None] * HPC for _ in range(NQT)]

            def emit_norm(h, qt, po):
                # reciprocal_approx_fast misreads PSUM sources on HW — bounce
                # the denominator row through SBUF first.
                dr0 = dpool.tile([1, QT], F32, tag="dr0", name=f"dr0_{h}_{qt}")
                nc.vector.tensor_copy(dr0[:], po[64:65, :])
                dr = dpool.tile([1, QT], F32, tag="dr", name=f"dr{h}_{qt}")
                nc.vector.reciprocal_approx_fast(out=dr[:], in_=dr0[:])
                db = dpool.tile([64, QT], F32, tag="db", name=f"db{h}_{qt}")
                nc.gpsimd.partition_broadcast(db[:], dr[:])
                oh = opool.tile([64, QT], BF16, tag="O", name=f"O{h}_{qt}")
                nc.vector.tensor_mul(oh[:], po[0:64, :], db[:])
                O[qt][h] = oh

            def emit_attn(h, qt, filler=None, interleave=False):
                """One (head, q-tile): an uninterrupted S-pair run (keeps the
                PE in 64-row mode with LDW hidden by parity alternation), then
                the AV block; buffer pressure paces both against ACT. The
                final tile interleaves AV so the kernel tail is short."""
                po = accpool.tile([65, QT], F32, tag="o", bufs=2, name=f"po{h}_{qt}")
                if interleave:
                    for g in range(ngrp):
                        emit_s_group(h, qt, g)
                        if g == 1 and filler is not None:
                            filler()
                        if g >= 2:
                            emit_av_group(h, qt, g - 2, po)
                    emit_av_group(h, qt, ngrp - 2, po)
                    emit_av_group(h, qt, ngrp - 1, po)
                    emit_norm(h, qt, po)
                    return
                for g in range(ngrp):
                    emit_s_group(h, qt, g)
                if filler is not None:
                    filler()
                for g in range(ngrp):
                    emit_av_group(h, qt, g, po)
                emit_norm(h, qt, po)

            def emit_proj(qt):
                for oc in range(CCH):
                    py = accpool.tile([128, QT], F32, tag="yb", name=f"py{qt}_{oc}")
                    for h in range(HPC):
                        nc.tensor.matmul(
                            py[:],
                            lhsT=wp_sb[0:64, h * DIM + oc * 128: h * DIM + (oc + 1) * 128],
                            rhs=O[qt][h][:],
                            start=(h == 0),
                            stop=(h == HPC - 1),
                        )
                    ysb = ypool.tile([128, QT], F32, tag="y", name=f"y{qt}_{oc}")
                    nc.vector.tensor_copy(ysb[:], py[:])
                    nc.sync.dma_start(
                        out=yT[oc * 128:(oc + 1) * 128, qt * QT:(qt + 1) * QT],
                        in_=ysb[:],
                    )

            # ---- static schedule ----
            # head 0 qk tiles emitted incrementally with qt0's S groups: the
            # first exp fires after just two qk tiles instead of all eight.
            emit_qk_tile(0, 0)
            emit_qk_dup(0, 0)
            emit_qk_tile(0, 1)
            emit_qk_dup(0, 1)
            load_w2()
            for j in range(2, NQT):
                emit_s_group(0, 0, 2 * (j - 2))
                emit_s_group(0, 0, 2 * (j - 2) + 1)
                emit_qk_tile(0, j)
                emit_qk_dup(0, j)
            for g in range(12, ngrp):
                emit_s_group(0, 0, g)

            # v while ACT chews on qt0's exps, then the deferred AV for qt0.
            po00 = accpool.tile([65, QT], F32, tag="o", bufs=2, name="po0_0")
            for tt in range(NKB):
                emit_v_tile(tt)
            for g in range(ngrp):
                emit_av_group(0, 0, g, po00)
            emit_norm(0, 0, po00)

            def qk_filler(nexth, qt):
                if qt < NQT - 2:
                    return lambda: emit_qk_tile(nexth, qt - 1)
                if qt == NQT - 2:
                    def bulk():
                        for t in (NQT - 3, NQT - 2, NQT - 1):
                            emit_qk_tile(nexth, t)
                        emit_qk_dup(nexth)

                    return bulk
                return None

            for qt in range(1, NQT):
                emit_attn(0, qt, filler=qk_filler(1, qt))

            for qt in range(NQT):
                filler = qk_filler(2, qt) if qt >= 1 else None
                emit_attn(1, qt, filler=filler)

            for qt in range(NQT):
                filler = (lambda q=qt: emit_proj(q - 1)) if qt >= 1 else None
                emit_attn(2, qt, filler=filler, interleave=(qt == NQT - 1))
            emit_proj(NQT - 1)

    nc.compile()
    return nc


def make_in_maps(x, w_qkv):
    """Build the 8 per-core input maps from the full fp32 inputs."""
    bf = ml_dtypes.bfloat16
    in_maps = []
    for core in range(NCORES):
        b = core // 4
        hs = [(core % 4) * HPC + i for i in range(HPC)]
        xTb = np.ascontiguousarray(np.asarray(x[b]).T).astype(bf)
        wqk = np.empty((DIM, HPC * 128), dtype=bf)
        wv = np.empty((DIM, HPC * 64), dtype=bf)
        for i, h in enumerate(hs):
            wqk[:, i * 128: i * 128 + 64] = w_qkv[h * 64:(h + 1) * 64, :].T
            wqk[:, i * 128 + 64: i * 128 + 128] = w_qkv[DIM + h * 64: DIM + (h + 1) * 64, :].T
            wv[:, i * 64:(i + 1) * 64] = w_qkv[2 * DIM + h * 64: 2 * DIM + (h + 1) * 64, :].T
        in_maps.append({"xT": xTb, "wqkT": wqk, "wvT": wv})
    return in_maps


def make_wp_map(core, w_proj):
    bf = ml_dtypes.bfloat16
    hs = [(core % 4) * HPC + i for i in range(HPC)]
    wp = np.empty((HPC * 64, DIM), dtype=bf)
    for i, h in enumerate(hs):
        wp[i * 64:(i + 1) * 64, :] = w_proj[:, h * 64:(h + 1) * 64].T
    return wp


_NC = None


def kernel(x, w_qkv, w_proj, b_proj):
    global _NC
    if _NC is None:
        _NC = build_program()
    x = np.asarray(x, dtype=np.float32)
    w_qkv = np.asarray(w_qkv, dtype=np.float32)
    w_proj = np.asarray(w_proj, dtype=np.float32)
    b_proj = np.asarray(b_proj, dtype=np.float32)

    in_maps = make_in_maps(x, w_qkv)
    for core in range(NCORES):
        in_maps[core]["wpT"] = make_wp_map(core, w_proj)

    r = run_bass_kernel_spmd(_NC, in_maps, list(range(NCORES)))
    y = np.zeros((B, N, DIM), dtype=np.float32)
    for core in range(NCORES):
        b = core // 4
        y[b] += r.results[core]["yT"].T
    y += b_proj[None, None, :]
    return y


# revision 26
# speedup vs baseline: 1.1015x; 1.0292x over previous
"""Trainium2 Bass kernel for nn_Attention (B=2, N=4096, DIM=768, H=12 heads).

Sharding: 24 (batch, head) pairs over 8 cores -> 3 heads per core, 4 cores
per batch element. Each core computes, for its batch b and its 3 heads:
  q,k,v projections -> full attention (flash-style, no score materialization
  to HBM) -> partial output projection  y_partial^T = sum_h wp_h^T @ O_h^T.
The host sums the 4 partials per batch and adds the bias.

Device dataflow (all matmul inputs bf16, fp32 PSUM accumulation):
  - host passes x^T, w_qk^T, w_v^T, w_p^T pre-transposed/pre-sliced in bf16
  - qk^T = [w_q|w_k]^T.T @ x^T    -> q^T,k^T [64, 4096] per head (d-major)
  - S^T[k,q] = k^T.T @ q^T        -> PSUM, 2-way row-packed (K=64)
  - P^T = exp(S^T * scale)        -> ScalarE (the bottleneck engine)
  - O^T|den = [V|1].T @ P^T       -> PSUM accumulate over k blocks
  - O^T /= den (recip + gpsimd partition-broadcast + DVE mult)
  - y^T += wp_h^T.T @ O_h^T       -> per-head K=64 accumulation
"""

import numpy as np
import ml_dtypes

import concourse.bacc as bacc
import concourse.mybir as mybir
import concourse.tile as tile
from concourse.bass_utils import run_bass_kernel_spmd

BF16 = mybir.dt.bfloat16
F32 = mybir.dt.float32

DIM = 768
N = 4096
NUM_HEADS = 12
HEAD_DIM = 64
SCALE = HEAD_DIM ** -0.5
B = 2
NCORES = 8
HPC = 3  # heads per core
CCH = DIM // 128  # 6 contraction chunks of 128
NQT = 8  # q tiles of 512
QT = 512
NKB = 32  # k blocks of 128
KB = 128
GRP = 2  # k-blocks per exp group (2 PSUM banks, aligns with row-pack pairs)


def build_program():
    nc = bacc.Bacc("TRN2", target_bir_lowering=False, debug=False)

    xT = nc.dram_tensor("xT", [DIM, N], BF16, kind="ExternalInput")
    wqkT = nc.dram_tensor("wqkT", [DIM, HPC * 128], BF16, kind="ExternalInput")
    wvT = nc.dram_tensor("wvT", [DIM, HPC * 64], BF16, kind="ExternalInput")
    wpT = nc.dram_tensor("wpT", [HPC * 64, DIM], BF16, kind="ExternalInput")
    yT = nc.dram_tensor("yT", [DIM, N], F32, kind="ExternalOutput")

    ngrp = NKB // GRP

    with tile.TileContext(nc) as tc:
        with (
            tc.tile_pool(name="wpool", bufs=1) as wpool,
            tc.tile_pool(name="qkpool", bufs=1) as qkpool,
            tc.tile_pool(name="pspool", bufs=2, space="PSUM") as pspool,
            tc.tile_pool(name="vpool", bufs=1, space="PSUM") as vpool,
            tc.tile_pool(name="accpool", bufs=1, space="PSUM") as accpool,
            tc.tile_pool(name="espool", bufs=20) as espool,
            tc.tile_pool(name="dpool", bufs=3) as dpool,
            tc.tile_pool(name="opool", bufs=19) as opool,
            tc.tile_pool(name="ypool", bufs=4) as ypool,
        ):
            # xT split into one tile per q-token-tile so phase A can start as
            # soon as the first slice lands (per-tile DMA dependencies).
            xTs = [
                wpool.tile([128, CCH * QT], BF16, tag=f"xT{qt}", name=f"xT{qt}")
                for qt in range(NQT)
            ]
            wqk_sb = wpool.tile([128, CCH * HPC * 128], BF16, tag="wqk")
            wv_sb = wpool.tile([128, CCH * HPC * 64], BF16, tag="wv")
            wp_sb = wpool.tile([64, HPC * DIM], BF16, tag="wp")
            T = [
                qkpool.tile([128, 2 * N], BF16, tag=f"T{h}", name=f"T{h}")
                for h in range(HPC)
            ]
            V = [
                qkpool.tile([128, NKB * 65], BF16, tag=f"V{h}", name=f"V{h}")
                for h in range(HPC)
            ]

            # consolidated multi-dim DMAs: one instruction per destination
            # tile keeps the Sync sequencer FIFO short (it issues serially).
            wqk_src = wqkT[:].rearrange("(c p) n -> p c n", p=128)
            nc.sync.dma_start(
                out=wqk_sb[:].rearrange("p (c n) -> p c n", n=384), in_=wqk_src
            )
            xT_src = xT[:].rearrange("(c p) n -> p c n", p=128)
            xT_loaded = set()

            def load_xT(qt):
                if qt in xT_loaded:
                    return
                xT_loaded.add(qt)
                nc.sync.dma_start(
                    out=xTs[qt][:].rearrange("p (c n) -> p c n", n=QT),
                    in_=xT_src[:, :, qt * QT:(qt + 1) * QT],
                )
            def load_w2():
                nc.sync.dma_start(
                    out=wv_sb[:].rearrange("p (c n) -> p c n", n=192),
                    in_=wvT[:].rearrange("(c p) n -> p c n", p=128),
                )
                nc.sync.dma_start(
                    out=wp_sb[0:64, :].rearrange("p (h n) -> p h n", n=DIM),
                    in_=wpT[:].rearrange("(h p) n -> p h n", p=64),
                )
            for h in range(HPC):
                nc.gpsimd.memset(V[h][:], 1.0)

            # ---- emission helpers ----
            def emit_qk_tile(h, qt):
                load_xT(qt)
                ps = pspool.tile([128, QT], F32, tag="s", name=f"qk{h}_{qt}")
                for c in range(CCH):
                    nc.tensor.matmul(
                        ps[:],
                        lhsT=wqk_sb[:, c * 384 + h * 128: c * 384 + (h + 1) * 128],
                        rhs=xTs[qt][:, c * QT:(c + 1) * QT],
                        start=(c == 0),
                        stop=(c == CCH - 1),
                    )
                nc.vector.tensor_copy(
                    T[h][0:64, N + qt * QT: N + (qt + 1) * QT], ps[0:64, :]
                )
                nc.vector.tensor_copy(
                    T[h][64:128, qt * QT:(qt + 1) * QT], ps[64:128, :]
                )

            def emit_qk_dup(h, qt=None):
                if qt is None:
                    nc.sync.dma_start(out=T[h][0:64, 0:N], in_=T[h][64:128, 0:N])
                    nc.sync.dma_start(
                        out=T[h][64:128, N:2 * N], in_=T[h][0:64, N:2 * N]
                    )
                else:
                    s = qt * QT
                    nc.sync.dma_start(
                        out=T[h][0:64, s:s + QT], in_=T[h][64:128, s:s + QT]
                    )
                    nc.sync.dma_start(
                        out=T[h][64:128, N + s: N + s + QT],
                        in_=T[h][0:64, N + s: N + s + QT],
                    )

            def emit_v_tile(tt):
                ps = vpool.tile([128, HPC * 64], F32, tag="v", name=f"v{tt}")
                for c in range(CCH):
                    nc.tensor.matmul(
                        ps[:],
                        lhsT=xTs[tt // 4][:, c * QT + (tt % 4) * 128: c * QT + (tt % 4) * 128 + 128],
                        rhs=wv_sb[:, c * 192:(c + 1) * 192],
                        start=(c == 0),
                        stop=(c == CCH - 1),
                    )
                for h in range(HPC):
                    nc.vector.tensor_copy(
                        V[h][:, tt * 65: tt * 65 + 64],
                        ps[:, h * 64:(h + 1) * 64],
                    )

            es_store = {}

            def emit_s_group(h, qt, g):
                ps = pspool.tile([128, GRP * QT], F32, tag="s", name=f"ps{h}_{qt}_{g}")
                es = espool.tile([128, GRP * QT], BF16, tag="es", name=f"es{h}_{qt}_{g}")
                for j in range(GRP):
                    kb = g * GRP + j
                    o = 64 * (kb % 2)
                    nc.tensor.matmul(
                        ps[:, j * QT:(j + 1) * QT],
                        lhsT=T[h][o:o + 64, kb * KB:(kb + 1) * KB],
                        rhs=T[h][o:o + 64, N + qt * QT: N + (qt + 1) * QT],
                        start=True,
                        stop=True,
                    )
                nc.scalar.activation(
                    es[:], ps[:], mybir.ActivationFunctionType.Exp, scale=SCALE
                )
                es_store[(h, qt, g)] = es

            def emit_av_group(h, qt, g, po):
                es = es_store.pop((h, qt, g))
                for j in range(GRP):
                    kb = g * GRP + j
                    nc.tensor.matmul(
                        po[:],
                        lhsT=V[h][:, kb * 65: kb * 65 + 65],
                        rhs=es[:, j * QT:(j + 1) * QT],
                        start=(kb == 0),
                        stop=(kb == NKB - 1),
                        skip_group_check=True,
                    )

            O = [[None] * HPC for _ in range(NQT)]

            def emit_norm(h, qt, po):
                # reciprocal_approx_fast misreads PSUM sources on HW — bounce
                # the denominator row through SBUF first.
                dr0 = dpool.tile([1, QT], F32, tag="dr0", name=f"dr0_{h}_{qt}")
                nc.vector.tensor_copy(dr0[:], po[64:65, :])
                dr = dpool.tile([1, QT], F32, tag="dr", name=f"dr{h}_{qt}")
                nc.vector.reciprocal_approx_fast(out=dr[:], in_=dr0[:])
                db = dpool.tile([64, QT], F32, tag="db", name=f"db{h}_{qt}")
                nc.gpsimd.partition_broadcast(db[:], dr[:])
                oh = opool.tile([64, QT], BF16, tag="O", name=f"O{h}_{qt}")
                nc.vector.tensor_mul(oh[:], po[0:64, :], db[:])
                O[qt][h] = oh

            def emit_attn(h, qt, filler=None, interleave=False, pre=0):
                """One (head, q-tile): an uninterrupted S-pair run (keeps the
                PE in 64-row mode with LDW hidden by parity alternation), then
                the AV block; buffer pressure paces both against ACT. The
                final tile interleaves AV so the kernel tail is short."""
                po = accpool.tile([65, QT], F32, tag="o", bufs=2, name=f"po{h}_{qt}")
                if interleave:
                    for g in range(ngrp):
                        emit_s_group(h, qt, g)
                        if g == 1 and filler is not None:
                            filler()
                        if g >= 2:
                            emit_av_group(h, qt, g - 2, po)
                    emit_av_group(h, qt, ngrp - 2, po)
                    emit_av_group(h, qt, ngrp - 1, po)
                    emit_norm(h, qt, po)
                    return
                for g in range(pre, ngrp):
                    emit_s_group(h, qt, g)
                if filler is not None:
                    filler()
                for g in range(ngrp):
                    emit_av_group(h, qt, g, po)
                emit_norm(h, qt, po)

            def emit_proj(qt):
                for oc in range(CCH):
                    pool, tg = (accpool, "yb") if oc % 2 == 0 else (vpool, "v")
                    py = pool.tile([128, QT], F32, tag=tg, name=f"py{qt}_{oc}")
                    for h in range(HPC):
                        nc.tensor.matmul(
                            py[:],
                            lhsT=wp_sb[0:64, h * DIM + oc * 128: h * DIM + (oc + 1) * 128],
                            rhs=O[qt][h][:],
                            start=(h == 0),
                            stop=(h == HPC - 1),
                        )
                    ysb = ypool.tile([128, QT], F32, tag="y", name=f"y{qt}_{oc}")
                    nc.vector.tensor_copy(ysb[:], py[:])
                    nc.sync.dma_start(
                        out=yT[oc * 128:(oc + 1) * 128, qt * QT:(qt + 1) * QT],
                        in_=ysb[:],
                    )

            # ---- static schedule ----
            # head 0 qk tiles emitted incrementally with qt0's S groups: the
            # first exp fires after just two qk tiles instead of all eight.
            emit_qk_tile(0, 0)
            emit_qk_dup(0, 0)
            emit_qk_tile(0, 1)
            emit_qk_dup(0, 1)
            load_w2()
            for j in range(2, NQT):
                emit_s_group(0, 0, 2 * (j - 2))
                emit_s_group(0, 0, 2 * (j - 2) + 1)
                emit_qk_tile(0, j)
                emit_qk_dup(0, j)
            for g in range(12, ngrp):
                emit_s_group(0, 0, g)

            # v while ACT chews on qt0's exps, then the deferred AV for qt0.
            po00 = accpool.tile([65, QT], F32, tag="o", bufs=2, name="po0_0")
            for tt in range(NKB):
                emit_v_tile(tt)
            for g in range(4):
                emit_s_group(0, 1, g)
            for g in range(ngrp):
                emit_av_group(0, 0, g, po00)
            emit_norm(0, 0, po00)

            def qk_filler(nexth, qt):
                if qt < NQT - 2:
                    return lambda: emit_qk_tile(nexth, qt - 1)
                if qt == NQT - 2:
                    def bulk():
                        for t in (NQT - 3, NQT - 2, NQT - 1):
                            emit_qk_tile(nexth, t)
                        emit_qk_dup(nexth)

                    return bulk
                return None

            for qt in range(1, NQT):
                emit_attn(0, qt, filler=qk_filler(1, qt), pre=(4 if qt == 1 else 0))

            for qt in range(NQT):
                filler = qk_filler(2, qt) if qt >= 1 else None
                emit_attn(1, qt, filler=filler)

            for qt in range(NQT):
                filler = (lambda q=qt: emit_proj(q - 1)) if qt >= 1 else None
                emit_attn(2, qt, filler=filler, interleave=(qt == NQT - 1))
            emit_proj(NQT - 1)

    nc.compile()
    return nc


def make_in_maps(x, w_qkv):
    """Build the 8 per-core input maps from the full fp32 inputs."""
    bf = ml_dtypes.bfloat16
    in_maps = []
    for core in range(NCORES):
        b = core // 4
        hs = [(core % 4) * HPC + i for i in range(HPC)]
        xTb = np.ascontiguousarray(np.asarray(x[b]).T).astype(bf)
        wqk = np.empty((DIM, HPC * 128), dtype=bf)
        wv = np.empty((DIM, HPC * 64), dtype=bf)
        for i, h in enumerate(hs):
            wqk[:, i * 128: i * 128 + 64] = w_qkv[h * 64:(h + 1) * 64, :].T
            wqk[:, i * 128 + 64: i * 128 + 128] = w_qkv[DIM + h * 64: DIM + (h + 1) * 64, :].T
            wv[:, i * 64:(i + 1) * 64] = w_qkv[2 * DIM + h * 64: 2 * DIM + (h + 1) * 64, :].T
        in_maps.append({"xT": xTb, "wqkT": wqk, "wvT": wv})
    return in_maps


def make_wp_map(core, w_proj):
    bf = ml_dtypes.bfloat16
    hs = [(core % 4) * HPC + i for i in range(HPC)]
    wp = np.empty((HPC * 64, DIM), dtype=bf)
    for i, h in enumerate(hs):
        wp[i * 64:(i + 1) * 64, :] = w_proj[:, h * 64:(h + 1) * 64].T
    return wp


_NC = None


def kernel(x, w_qkv, w_proj, b_proj):
    global _NC
    if _NC is None:
        _NC = build_program()
    x = np.asarray(x, dtype=np.float32)
    w_qkv = np.asarray(w_qkv, dtype=np.float32)
    w_proj = np.asarray(w_proj, dtype=np.float32)
    b_proj = np.asarray(b_proj, dtype=np.float32)

    in_maps = make_in_maps(x, w_qkv)
    for core in range(NCORES):
        in_maps[core]["wpT"] = make_wp_map(core, w_proj)

    r = run_bass_kernel_spmd(_NC, in_maps, list(range(NCORES)))
    y = np.zeros((B, N, DIM), dtype=np.float32)
    for core in range(NCORES):
        b = core // 4
        y[b] += r.results[core]["yT"].T
    y += b_proj[None, None, :]
    return y


# revision 27
# speedup vs baseline: 1.1145x; 1.0118x over previous
"""Trainium2 Bass kernel for nn_Attention (B=2, N=4096, DIM=768, H=12 heads).

Sharding: 24 (batch, head) pairs over 8 cores -> 3 heads per core, 4 cores
per batch element. Each core computes, for its batch b and its 3 heads:
  q,k,v projections -> full attention (flash-style, no score materialization
  to HBM) -> partial output projection  y_partial^T = sum_h wp_h^T @ O_h^T.
The host sums the 4 partials per batch and adds the bias.

Device dataflow (all matmul inputs bf16, fp32 PSUM accumulation):
  - host passes x^T, w_qk^T, w_v^T, w_p^T pre-transposed/pre-sliced in bf16
  - qk^T = [w_q|w_k]^T.T @ x^T    -> q^T,k^T [64, 4096] per head (d-major)
  - S^T[k,q] = k^T.T @ q^T        -> PSUM, 2-way row-packed (K=64)
  - P^T = exp(S^T * scale)        -> ScalarE (the bottleneck engine)
  - O^T|den = [V|1].T @ P^T       -> PSUM accumulate over k blocks
  - O^T /= den (recip + gpsimd partition-broadcast + DVE mult)
  - y^T += wp_h^T.T @ O_h^T       -> per-head K=64 accumulation
"""

import numpy as np
import ml_dtypes

import concourse.bacc as bacc
import concourse.mybir as mybir
import concourse.tile as tile
from concourse.bass_utils import run_bass_kernel_spmd

BF16 = mybir.dt.bfloat16
F32 = mybir.dt.float32

DIM = 768
N = 4096
NUM_HEADS = 12
HEAD_DIM = 64
SCALE = HEAD_DIM ** -0.5
B = 2
NCORES = 8
HPC = 3  # heads per core
CCH = DIM // 128  # 6 contraction chunks of 128
NQT = 8  # q tiles of 512
QT = 512
NKB = 32  # k blocks of 128
KB = 128
GRP = 2  # k-blocks per exp group (2 PSUM banks, aligns with row-pack pairs)


def build_program():
    nc = bacc.Bacc("TRN2", target_bir_lowering=False, debug=False)

    xT = nc.dram_tensor("xT", [DIM, N], BF16, kind="ExternalInput")
    wqkT = nc.dram_tensor("wqkT", [DIM, HPC * 128], BF16, kind="ExternalInput")
    wvT = nc.dram_tensor("wvT", [DIM, HPC * 64], BF16, kind="ExternalInput")
    wpT = nc.dram_tensor("wpT", [HPC * 64, DIM], BF16, kind="ExternalInput")
    yT = nc.dram_tensor("yT", [DIM, N], F32, kind="ExternalOutput")

    ngrp = NKB // GRP

    with tile.TileContext(nc) as tc:
        with (
            tc.tile_pool(name="wpool", bufs=1) as wpool,
            tc.tile_pool(name="qkpool", bufs=1) as qkpool,
            tc.tile_pool(name="pspool", bufs=2, space="PSUM") as pspool,
            tc.tile_pool(name="vpool", bufs=1, space="PSUM") as vpool,
            tc.tile_pool(name="accpool", bufs=1, space="PSUM") as accpool,
            tc.tile_pool(name="espool", bufs=20) as espool,
            tc.tile_pool(name="dpool", bufs=3) as dpool,
            tc.tile_pool(name="opool", bufs=19) as opool,
            tc.tile_pool(name="ypool", bufs=4) as ypool,
        ):
            # xT split into one tile per q-token-tile so phase A can start as
            # soon as the first slice lands (per-tile DMA dependencies).
            xTs = [
                wpool.tile([128, CCH * QT], BF16, tag=f"xT{qt}", name=f"xT{qt}")
                for qt in range(NQT)
            ]
            wqk_sb = wpool.tile([128, CCH * HPC * 128], BF16, tag="wqk")
            wv_sb = wpool.tile([128, CCH * HPC * 64], BF16, tag="wv")
            wp_sb = wpool.tile([64, HPC * DIM], BF16, tag="wp")
            T = [
                qkpool.tile([128, 2 * N], BF16, tag=f"T{h}", name=f"T{h}")
                for h in range(HPC)
            ]
            V = [
                qkpool.tile([128, NKB * 65], BF16, tag=f"V{h}", name=f"V{h}")
                for h in range(HPC)
            ]

            # consolidated multi-dim DMAs: one instruction per destination
            # tile keeps the Sync sequencer FIFO short (it issues serially).
            wqk_src = wqkT[:].rearrange("(c p) n -> p c n", p=128)
            nc.sync.dma_start(
                out=wqk_sb[:].rearrange("p (c n) -> p c n", n=384), in_=wqk_src
            )
            xT_src = xT[:].rearrange("(c p) n -> p c n", p=128)
            xT_loaded = set()

            def load_xT(qt):
                if qt in xT_loaded:
                    return
                xT_loaded.add(qt)
                nc.sync.dma_start(
                    out=xTs[qt][:].rearrange("p (c n) -> p c n", n=QT),
                    in_=xT_src[:, :, qt * QT:(qt + 1) * QT],
                )
            def load_w2():
                nc.sync.dma_start(
                    out=wv_sb[:].rearrange("p (c n) -> p c n", n=192),
                    in_=wvT[:].rearrange("(c p) n -> p c n", p=128),
                )
                nc.sync.dma_start(
                    out=wp_sb[0:64, :].rearrange("p (h n) -> p h n", n=DIM),
                    in_=wpT[:].rearrange("(h p) n -> p h n", p=64),
                )
            for h in range(HPC):
                nc.gpsimd.memset(V[h][:], 1.0)

            # ---- emission helpers ----
            def emit_qk_tile(h, qt):
                load_xT(qt)
                ps = pspool.tile([128, QT], F32, tag="s", name=f"qk{h}_{qt}")
                for c in range(CCH):
                    nc.tensor.matmul(
                        ps[:],
                        lhsT=wqk_sb[:, c * 384 + h * 128: c * 384 + (h + 1) * 128],
                        rhs=xTs[qt][:, c * QT:(c + 1) * QT],
                        start=(c == 0),
                        stop=(c == CCH - 1),
                    )
                nc.vector.tensor_copy(
                    T[h][0:64, N + qt * QT: N + (qt + 1) * QT], ps[0:64, :]
                )
                nc.vector.tensor_copy(
                    T[h][64:128, qt * QT:(qt + 1) * QT], ps[64:128, :]
                )

            def emit_qk_dup(h, qt=None):
                if qt is None:
                    nc.sync.dma_start(out=T[h][0:64, 0:N], in_=T[h][64:128, 0:N])
                    nc.sync.dma_start(
                        out=T[h][64:128, N:2 * N], in_=T[h][0:64, N:2 * N]
                    )
                else:
                    s = qt * QT
                    nc.sync.dma_start(
                        out=T[h][0:64, s:s + QT], in_=T[h][64:128, s:s + QT]
                    )
                    nc.sync.dma_start(
                        out=T[h][64:128, N + s: N + s + QT],
                        in_=T[h][0:64, N + s: N + s + QT],
                    )

            def emit_v_tile(tt):
                ps = vpool.tile([128, HPC * 64], F32, tag="v", name=f"v{tt}")
                for c in range(CCH):
                    nc.tensor.matmul(
                        ps[:],
                        lhsT=xTs[tt // 4][:, c * QT + (tt % 4) * 128: c * QT + (tt % 4) * 128 + 128],
                        rhs=wv_sb[:, c * 192:(c + 1) * 192],
                        start=(c == 0),
                        stop=(c == CCH - 1),
                    )
                for h in range(HPC):
                    nc.vector.tensor_copy(
                        V[h][:, tt * 65: tt * 65 + 64],
                        ps[:, h * 64:(h + 1) * 64],
                    )

            es_store = {}

            def emit_s_group(h, qt, g):
                ps = pspool.tile([128, GRP * QT], F32, tag="s", name=f"ps{h}_{qt}_{g}")
                es = espool.tile([128, GRP * QT], BF16, tag="es", name=f"es{h}_{qt}_{g}")
                for j in range(GRP):
                    kb = g * GRP + j
                    o = 64 * (kb % 2)
                    nc.tensor.matmul(
                        ps[:, j * QT:(j + 1) * QT],
                        lhsT=T[h][o:o + 64, kb * KB:(kb + 1) * KB],
                        rhs=T[h][o:o + 64, N + qt * QT: N + (qt + 1) * QT],
                        start=True,
                        stop=True,
                    )
                nc.scalar.activation(
                    es[:], ps[:], mybir.ActivationFunctionType.Exp, scale=SCALE
                )
                es_store[(h, qt, g)] = es

            def emit_av_group(h, qt, g, po):
                es = es_store.pop((h, qt, g))
                for j in range(GRP):
                    kb = g * GRP + j
                    nc.tensor.matmul(
                        po[:],
                        lhsT=V[h][:, kb * 65: kb * 65 + 65],
                        rhs=es[:, j * QT:(j + 1) * QT],
                        start=(kb == 0),
                        stop=(kb == NKB - 1),
                        skip_group_check=True,
                    )

            O = [[None] * HPC for _ in range(NQT)]

            def emit_norm(h, qt, po):
                # reciprocal_approx_fast misreads PSUM sources on HW — bounce
                # the denominator row through SBUF first.
                dr0 = dpool.tile([1, QT], F32, tag="dr0", name=f"dr0_{h}_{qt}")
                nc.vector.tensor_copy(dr0[:], po[64:65, :])
                dr = dpool.tile([1, QT], F32, tag="dr", name=f"dr{h}_{qt}")
                nc.vector.reciprocal_approx_fast(out=dr[:], in_=dr0[:])
                db = dpool.tile([64, QT], F32, tag="db", name=f"db{h}_{qt}")
                nc.gpsimd.partition_broadcast(db[:], dr[:])
                oh = opool.tile([64, QT], BF16, tag="O", name=f"O{h}_{qt}")
                nc.vector.tensor_mul(oh[:], po[0:64, :], db[:])
                O[qt][h] = oh

            def emit_attn(h, qt, filler=None, interleave=False, pre=0,
                          pre_next=None):
                """One (head, q-tile): an uninterrupted S-pair run (keeps the
                PE in 64-row mode with LDW hidden by parity alternation), then
                the AV block; buffer pressure paces both against ACT. The
                final tile interleaves AV so the kernel tail is short."""
                po = accpool.tile([65, QT], F32, tag="o", bufs=2, name=f"po{h}_{qt}")
                if interleave:
                    for g in range(pre, ngrp):
                        emit_s_group(h, qt, g)
                        if g == pre + 1 and filler is not None:
                            filler()
                        if g >= 2:
                            emit_av_group(h, qt, g - 2, po)
                    emit_av_group(h, qt, ngrp - 2, po)
                    emit_av_group(h, qt, ngrp - 1, po)
                    emit_norm(h, qt, po)
                    return
                for g in range(pre, ngrp):
                    emit_s_group(h, qt, g)
                if filler is not None:
                    filler()
                # pre-emit the first S groups of the NEXT tile so ACT keeps
                # chewing while the PE runs this tile's AV block + norm.
                if pre_next is not None:
                    for g in range(2):
                        emit_s_group(pre_next[0], pre_next[1], g)
                for g in range(ngrp):
                    emit_av_group(h, qt, g, po)
                emit_norm(h, qt, po)

            def emit_proj(qt):
                for oc in range(CCH):
                    pool, tg = (accpool, "yb") if oc % 2 == 0 else (vpool, "v")
                    py = pool.tile([128, QT], F32, tag=tg, name=f"py{qt}_{oc}")
                    for h in range(HPC):
                        nc.tensor.matmul(
                            py[:],
                            lhsT=wp_sb[0:64, h * DIM + oc * 128: h * DIM + (oc + 1) * 128],
                            rhs=O[qt][h][:],
                            start=(h == 0),
                            stop=(h == HPC - 1),
                        )
                    ysb = ypool.tile([128, QT], F32, tag="y", name=f"y{qt}_{oc}")
                    nc.vector.tensor_copy(ysb[:], py[:])
                    nc.sync.dma_start(
                        out=yT[oc * 128:(oc + 1) * 128, qt * QT:(qt + 1) * QT],
                        in_=ysb[:],
                    )

            # ---- static schedule ----
            # head 0 qk tiles emitted incrementally with qt0's S groups: the
            # first exp fires after just two qk tiles instead of all eight.
            emit_qk_tile(0, 0)
            emit_qk_dup(0, 0)
            emit_qk_tile(0, 1)
            emit_qk_dup(0, 1)
            load_w2()
            for j in range(2, NQT):
                emit_s_group(0, 0, 2 * (j - 2))
                emit_s_group(0, 0, 2 * (j - 2) + 1)
                emit_qk_tile(0, j)
                emit_qk_dup(0, j)
            for g in range(12, ngrp):
                emit_s_group(0, 0, g)

            # v while ACT chews on qt0's exps, then the deferred AV for qt0.
            po00 = accpool.tile([65, QT], F32, tag="o", bufs=2, name="po0_0")
            for tt in range(NKB):
                emit_v_tile(tt)
            for g in range(4):
                emit_s_group(0, 1, g)
            for g in range(ngrp):
                emit_av_group(0, 0, g, po00)
            emit_norm(0, 0, po00)

            def qk_filler(nexth, qt):
                if qt < NQT - 2:
                    return lambda: emit_qk_tile(nexth, qt - 1)
                if qt == NQT - 2:
                    def bulk():
                        for t in (NQT - 3, NQT - 2, NQT - 1):
                            emit_qk_tile(nexth, t)
                        emit_qk_dup(nexth)

                    return bulk
                return None

            for qt in range(1, NQT):
                nxt = (0, qt + 1) if qt < NQT - 1 else (1, 0)
                emit_attn(
                    0, qt, filler=qk_filler(1, qt),
                    pre=(4 if qt == 1 else 2), pre_next=nxt,
                )

            for qt in range(NQT):
                filler = qk_filler(2, qt) if qt >= 1 else None
                nxt = (1, qt + 1) if qt < NQT - 1 else (2, 0)
                emit_attn(1, qt, filler=filler, pre=2, pre_next=nxt)

            for qt in range(NQT):
                filler = (lambda q=qt: emit_proj(q - 1)) if qt >= 1 else None
                nxt = (2, qt + 1) if qt < NQT - 1 else None
                emit_attn(
                    2, qt, filler=filler, interleave=(qt == NQT - 1),
                    pre=2, pre_next=nxt,
                )
            emit_proj(NQT - 1)

    nc.compile()
    return nc


def make_in_maps(x, w_qkv):
    """Build the 8 per-core input maps from the full fp32 inputs."""
    bf = ml_dtypes.bfloat16
    in_maps = []
    for core in range(NCORES):
        b = core // 4
        hs = [(core % 4) * HPC + i for i in range(HPC)]
        xTb = np.ascontiguousarray(np.asarray(x[b]).T).astype(bf)
        wqk = np.empty((DIM, HPC * 128), dtype=bf)
        wv = np.empty((DIM, HPC * 64), dtype=bf)
        for i, h in enumerate(hs):
            wqk[:, i * 128: i * 128 + 64] = w_qkv[h * 64:(h + 1) * 64, :].T
            wqk[:, i * 128 + 64: i * 128 + 128] = w_qkv[DIM + h * 64: DIM + (h + 1) * 64, :].T
            wv[:, i * 64:(i + 1) * 64] = w_qkv[2 * DIM + h * 64: 2 * DIM + (h + 1) * 64, :].T
        in_maps.append({"xT": xTb, "wqkT": wqk, "wvT": wv})
    return in_maps


def make_wp_map(core, w_proj):
    bf = ml_dtypes.bfloat16
    hs = [(core % 4) * HPC + i for i in range(HPC)]
    wp = np.empty((HPC * 64, DIM), dtype=bf)
    for i, h in enumerate(hs):
        wp[i * 64:(i + 1) * 64, :] = w_proj[:, h * 64:(h + 1) * 64].T
    return wp


_NC = None


def kernel(x, w_qkv, w_proj, b_proj):
    global _NC
    if _NC is None:
        _NC = build_program()
    x = np.asarray(x, dtype=np.float32)
    w_qkv = np.asarray(w_qkv, dtype=np.float32)
    w_proj = np.asarray(w_proj, dtype=np.float32)
    b_proj = np.asarray(b_proj, dtype=np.float32)

    in_maps = make_in_maps(x, w_qkv)
    for core in range(NCORES):
        in_maps[core]["wpT"] = make_wp_map(core, w_proj)

    r = run_bass_kernel_spmd(_NC, in_maps, list(range(NCORES)))
    y = np.zeros((B, N, DIM), dtype=np.float32)
    for core in range(NCORES):
        b = core // 4
        y[b] += r.results[core]["yT"].T
    y += b_proj[None, None, :]
    return y


# revision 28
# speedup vs baseline: 1.1230x; 1.0076x over previous
"""Trainium2 Bass kernel for nn_Attention (B=2, N=4096, DIM=768, H=12 heads).

Sharding: 24 (batch, head) pairs over 8 cores -> 3 heads per core, 4 cores
per batch element. Each core computes, for its batch b and its 3 heads:
  q,k,v projections -> full attention (flash-style, no score materialization
  to HBM) -> partial output projection  y_partial^T = sum_h wp_h^T @ O_h^T.
The host sums the 4 partials per batch and adds the bias.

Device dataflow (all matmul inputs bf16, fp32 PSUM accumulation):
  - host passes x^T, w_qk^T, w_v^T, w_p^T pre-transposed/pre-sliced in bf16
  - qk^T = [w_q|w_k]^T.T @ x^T    -> q^T,k^T [64, 4096] per head (d-major)
  - S^T[k,q] = k^T.T @ q^T        -> PSUM, 2-way row-packed (K=64)
  - P^T = exp(S^T * scale)        -> ScalarE (the bottleneck engine)
  - O^T|den = [V|1].T @ P^T       -> PSUM accumulate over k blocks
  - O^T /= den (recip + gpsimd partition-broadcast + DVE mult)
  - y^T += wp_h^T.T @ O_h^T       -> per-head K=64 accumulation
"""

import numpy as np
import ml_dtypes

import concourse.bacc as bacc
import concourse.mybir as mybir
import concourse.tile as tile
from concourse.bass_utils import run_bass_kernel_spmd

BF16 = mybir.dt.bfloat16
F32 = mybir.dt.float32

DIM = 768
N = 4096
NUM_HEADS = 12
HEAD_DIM = 64
SCALE = HEAD_DIM ** -0.5
B = 2
NCORES = 8
HPC = 3  # heads per core
CCH = DIM // 128  # 6 contraction chunks of 128
NQT = 8  # q tiles of 512
QT = 512
NKB = 32  # k blocks of 128
KB = 128
GRP = 2  # k-blocks per exp group (2 PSUM banks, aligns with row-pack pairs)


def build_program():
    nc = bacc.Bacc("TRN2", target_bir_lowering=False, debug=False)

    xT = nc.dram_tensor("xT", [DIM, N], BF16, kind="ExternalInput")
    wqkT = nc.dram_tensor("wqkT", [DIM, HPC * 128], BF16, kind="ExternalInput")
    wvT = nc.dram_tensor("wvT", [DIM, HPC * 64], BF16, kind="ExternalInput")
    wpT = nc.dram_tensor("wpT", [HPC * 64, DIM], BF16, kind="ExternalInput")
    yT = nc.dram_tensor("yT", [DIM, N], F32, kind="ExternalOutput")

    ngrp = NKB // GRP

    with tile.TileContext(nc) as tc:
        with (
            tc.tile_pool(name="wpool", bufs=1) as wpool,
            tc.tile_pool(name="qkpool", bufs=1) as qkpool,
            tc.tile_pool(name="pspool", bufs=2, space="PSUM") as pspool,
            tc.tile_pool(name="vpool", bufs=1, space="PSUM") as vpool,
            tc.tile_pool(name="accpool", bufs=1, space="PSUM") as accpool,
            tc.tile_pool(name="espool", bufs=20) as espool,
            tc.tile_pool(name="dpool", bufs=3) as dpool,
            tc.tile_pool(name="opool", bufs=19) as opool,
            tc.tile_pool(name="ypool", bufs=4) as ypool,
        ):
            # xT split into one tile per q-token-tile so phase A can start as
            # soon as the first slice lands (per-tile DMA dependencies).
            xTs = [
                wpool.tile([128, CCH * QT], BF16, tag=f"xT{qt}", name=f"xT{qt}")
                for qt in range(NQT)
            ]
            wqk_sb = wpool.tile([128, CCH * HPC * 128], BF16, tag="wqk")
            wv_sb = wpool.tile([128, CCH * HPC * 64], BF16, tag="wv")
            wp_sb = wpool.tile([64, HPC * DIM], BF16, tag="wp")
            T = [
                qkpool.tile([128, 2 * N], BF16, tag=f"T{h}", name=f"T{h}")
                for h in range(HPC)
            ]
            V = [
                qkpool.tile([128, NKB * 65], BF16, tag=f"V{h}", name=f"V{h}")
                for h in range(HPC)
            ]

            # consolidated multi-dim DMAs: one instruction per destination
            # tile keeps the Sync sequencer FIFO short (it issues serially).
            wqk_src = wqkT[:].rearrange("(c p) n -> p c n", p=128)
            nc.sync.dma_start(
                out=wqk_sb[:].rearrange("p (c n) -> p c n", n=384), in_=wqk_src
            )
            xT_src = xT[:].rearrange("(c p) n -> p c n", p=128)
            xT_loaded = set()

            def load_xT(qt):
                if qt in xT_loaded:
                    return
                xT_loaded.add(qt)
                nc.sync.dma_start(
                    out=xTs[qt][:].rearrange("p (c n) -> p c n", n=QT),
                    in_=xT_src[:, :, qt * QT:(qt + 1) * QT],
                )
            def load_w2():
                nc.sync.dma_start(
                    out=wv_sb[:].rearrange("p (c n) -> p c n", n=192),
                    in_=wvT[:].rearrange("(c p) n -> p c n", p=128),
                )
                nc.sync.dma_start(
                    out=wp_sb[0:64, :].rearrange("p (h n) -> p h n", n=DIM),
                    in_=wpT[:].rearrange("(h p) n -> p h n", p=64),
                )
            for h in range(HPC):
                nc.gpsimd.memset(V[h][:], 1.0)

            # ---- emission helpers ----
            def emit_qk_tile(h, qt):
                load_xT(qt)
                ps = pspool.tile([128, QT], F32, tag="s", name=f"qk{h}_{qt}")
                for c in range(CCH):
                    nc.tensor.matmul(
                        ps[:],
                        lhsT=wqk_sb[:, c * 384 + h * 128: c * 384 + (h + 1) * 128],
                        rhs=xTs[qt][:, c * QT:(c + 1) * QT],
                        start=(c == 0),
                        stop=(c == CCH - 1),
                    )
                nc.vector.tensor_copy(
                    T[h][0:64, N + qt * QT: N + (qt + 1) * QT], ps[0:64, :]
                )
                nc.vector.tensor_copy(
                    T[h][64:128, qt * QT:(qt + 1) * QT], ps[64:128, :]
                )

            def emit_qk_dup(h, qt=None):
                if qt is None:
                    nc.sync.dma_start(out=T[h][0:64, 0:N], in_=T[h][64:128, 0:N])
                    nc.sync.dma_start(
                        out=T[h][64:128, N:2 * N], in_=T[h][0:64, N:2 * N]
                    )
                else:
                    s = qt * QT
                    nc.sync.dma_start(
                        out=T[h][0:64, s:s + QT], in_=T[h][64:128, s:s + QT]
                    )
                    nc.sync.dma_start(
                        out=T[h][64:128, N + s: N + s + QT],
                        in_=T[h][0:64, N + s: N + s + QT],
                    )

            def emit_v_tile(tt):
                ps = vpool.tile([128, HPC * 64], F32, tag="v", name=f"v{tt}")
                for c in range(CCH):
                    nc.tensor.matmul(
                        ps[:],
                        lhsT=xTs[tt // 4][:, c * QT + (tt % 4) * 128: c * QT + (tt % 4) * 128 + 128],
                        rhs=wv_sb[:, c * 192:(c + 1) * 192],
                        start=(c == 0),
                        stop=(c == CCH - 1),
                    )
                for h in range(HPC):
                    nc.vector.tensor_copy(
                        V[h][:, tt * 65: tt * 65 + 64],
                        ps[:, h * 64:(h + 1) * 64],
                    )

            es_store = {}

            def emit_s_group(h, qt, g):
                ps = pspool.tile([128, GRP * QT], F32, tag="s", name=f"ps{h}_{qt}_{g}")
                es = espool.tile([128, GRP * QT], BF16, tag="es", name=f"es{h}_{qt}_{g}")
                for j in range(GRP):
                    kb = g * GRP + j
                    o = 64 * (kb % 2)
                    nc.tensor.matmul(
                        ps[:, j * QT:(j + 1) * QT],
                        lhsT=T[h][o:o + 64, kb * KB:(kb + 1) * KB],
                        rhs=T[h][o:o + 64, N + qt * QT: N + (qt + 1) * QT],
                        start=True,
                        stop=True,
                    )
                nc.scalar.activation(
                    es[:], ps[:], mybir.ActivationFunctionType.Exp, scale=SCALE
                )
                es_store[(h, qt, g)] = es

            def emit_av_group(h, qt, g, po):
                es = es_store.pop((h, qt, g))
                for j in range(GRP):
                    kb = g * GRP + j
                    nc.tensor.matmul(
                        po[:],
                        lhsT=V[h][:, kb * 65: kb * 65 + 65],
                        rhs=es[:, j * QT:(j + 1) * QT],
                        start=(kb == 0),
                        stop=(kb == NKB - 1),
                        skip_group_check=True,
                    )

            O = [[None] * HPC for _ in range(NQT)]

            def emit_norm(h, qt, po):
                # reciprocal_approx_fast misreads PSUM sources on HW — bounce
                # the denominator row through SBUF first.
                dr0 = dpool.tile([1, QT], F32, tag="dr0", name=f"dr0_{h}_{qt}")
                nc.vector.tensor_copy(dr0[:], po[64:65, :])
                dr = dpool.tile([1, QT], F32, tag="dr", name=f"dr{h}_{qt}")
                nc.vector.reciprocal_approx_fast(out=dr[:], in_=dr0[:])
                db = dpool.tile([64, QT], F32, tag="db", name=f"db{h}_{qt}")
                nc.gpsimd.partition_broadcast(db[:], dr[:])
                oh = opool.tile([64, QT], BF16, tag="O", name=f"O{h}_{qt}")
                nc.vector.tensor_mul(oh[:], po[0:64, :], db[:])
                O[qt][h] = oh

            def emit_attn(h, qt, filler=None, interleave=False, pre=0,
                          pre_next=None):
                """One (head, q-tile): an uninterrupted S-pair run (keeps the
                PE in 64-row mode with LDW hidden by parity alternation), then
                the AV block; buffer pressure paces both against ACT. The
                final tile interleaves AV so the kernel tail is short."""
                po = accpool.tile([65, QT], F32, tag="o", bufs=2, name=f"po{h}_{qt}")
                if interleave:
                    for g in range(min(2, max(0, pre - 2))):
                        emit_av_group(h, qt, g, po)
                    for g in range(pre, ngrp):
                        emit_s_group(h, qt, g)
                        if g == pre + 1 and filler is not None:
                            filler()
                        if g >= 2:
                            emit_av_group(h, qt, g - 2, po)
                    emit_av_group(h, qt, ngrp - 2, po)
                    emit_av_group(h, qt, ngrp - 1, po)
                    emit_norm(h, qt, po)
                    return
                for g in range(pre, ngrp):
                    emit_s_group(h, qt, g)
                if filler is not None:
                    filler()
                # pre-emit the first S groups of the NEXT tile so ACT keeps
                # chewing while the PE runs this tile's AV block + norm.
                if pre_next is not None:
                    for g in range(4):
                        emit_s_group(pre_next[0], pre_next[1], g)
                for g in range(ngrp):
                    emit_av_group(h, qt, g, po)
                emit_norm(h, qt, po)

            def emit_proj(qt):
                for oc in range(CCH):
                    pool, tg = (accpool, "yb") if oc % 2 == 0 else (vpool, "v")
                    py = pool.tile([128, QT], F32, tag=tg, name=f"py{qt}_{oc}")
                    for h in range(HPC):
                        nc.tensor.matmul(
                            py[:],
                            lhsT=wp_sb[0:64, h * DIM + oc * 128: h * DIM + (oc + 1) * 128],
                            rhs=O[qt][h][:],
                            start=(h == 0),
                            stop=(h == HPC - 1),
                        )
                    ysb = ypool.tile([128, QT], F32, tag="y", name=f"y{qt}_{oc}")
                    nc.vector.tensor_copy(ysb[:], py[:])
                    nc.sync.dma_start(
                        out=yT[oc * 128:(oc + 1) * 128, qt * QT:(qt + 1) * QT],
                        in_=ysb[:],
                    )

            # ---- static schedule ----
            # head 0 qk tiles emitted incrementally with qt0's S groups: the
            # first exp fires after just two qk tiles instead of all eight.
            emit_qk_tile(0, 0)
            emit_qk_dup(0, 0)
            emit_qk_tile(0, 1)
            emit_qk_dup(0, 1)
            load_w2()
            for j in range(2, NQT):
                emit_s_group(0, 0, 2 * (j - 2))
                emit_s_group(0, 0, 2 * (j - 2) + 1)
                emit_qk_tile(0, j)
                emit_qk_dup(0, j)
            for g in range(12, ngrp):
                emit_s_group(0, 0, g)

            # v while ACT chews on qt0's exps, then the deferred AV for qt0.
            po00 = accpool.tile([65, QT], F32, tag="o", bufs=2, name="po0_0")
            for tt in range(NKB):
                emit_v_tile(tt)
            for g in range(4):
                emit_s_group(0, 1, g)
            for g in range(ngrp):
                emit_av_group(0, 0, g, po00)
            emit_norm(0, 0, po00)

            def qk_filler(nexth, qt):
                if qt < NQT - 2:
                    return lambda: emit_qk_tile(nexth, qt - 1)
                if qt == NQT - 2:
                    def bulk():
                        for t in (NQT - 3, NQT - 2, NQT - 1):
                            emit_qk_tile(nexth, t)
                        emit_qk_dup(nexth)

                    return bulk
                return None

            for qt in range(1, NQT):
                nxt = (0, qt + 1) if qt < NQT - 1 else (1, 0)
                emit_attn(
                    0, qt, filler=qk_filler(1, qt),
                    pre=4, pre_next=nxt,
                )

            for qt in range(NQT):
                filler = qk_filler(2, qt) if qt >= 1 else None
                nxt = (1, qt + 1) if qt < NQT - 1 else (2, 0)
                emit_attn(1, qt, filler=filler, pre=4, pre_next=nxt)

            for qt in range(NQT):
                filler = (lambda q=qt: emit_proj(q - 1)) if qt >= 1 else None
                nxt = (2, qt + 1) if qt < NQT - 1 else None
                emit_attn(
                    2, qt, filler=filler, interleave=(qt == NQT - 1),
                    pre=4, pre_next=nxt,
                )
            emit_proj(NQT - 1)

    nc.compile()
    return nc


def make_in_maps(x, w_qkv):
    """Build the 8 per-core input maps from the full fp32 inputs."""
    bf = ml_dtypes.bfloat16
    in_maps = []
    for core in range(NCORES):
        b = core // 4
        hs = [(core % 4) * HPC + i for i in range(HPC)]
        xTb = np.ascontiguousarray(np.asarray(x[b]).T).astype(bf)
        wqk = np.empty((DIM, HPC * 128), dtype=bf)
        wv = np.empty((DIM, HPC * 64), dtype=bf)
        for i, h in enumerate(hs):
            wqk[:, i * 128: i * 128 + 64] = w_qkv[h * 64:(h + 1) * 64, :].T
            wqk[:, i * 128 + 64: i * 128 + 128] = w_qkv[DIM + h * 64: DIM + (h + 1) * 64, :].T
            wv[:, i * 64:(i + 1) * 64] = w_qkv[2 * DIM + h * 64: 2 * DIM + (h + 1) * 64, :].T
        in_maps.append({"xT": xTb, "wqkT": wqk, "wvT": wv})
    return in_maps


def make_wp_map(core, w_proj):
    bf = ml_dtypes.bfloat16
    hs = [(core % 4) * HPC + i for i in range(HPC)]
    wp = np.empty((HPC * 64, DIM), dtype=bf)
    for i, h in enumerate(hs):
        wp[i * 64:(i + 1) * 64, :] = w_proj[:, h * 64:(h + 1) * 64].T
    return wp


_NC = None


def kernel(x, w_qkv, w_proj, b_proj):
    global _NC
    if _NC is None:
        _NC = build_program()
    x = np.asarray(x, dtype=np.float32)
    w_qkv = np.asarray(w_qkv, dtype=np.float32)
    w_proj = np.asarray(w_proj, dtype=np.float32)
    b_proj = np.asarray(b_proj, dtype=np.float32)

    in_maps = make_in_maps(x, w_qkv)
    for core in range(NCORES):
        in_maps[core]["wpT"] = make_wp_map(core, w_proj)

    r = run_bass_kernel_spmd(_NC, in_maps, list(range(NCORES)))
    y = np.zeros((B, N, DIM), dtype=np.float32)
    for core in range(NCORES):
        b = core // 4
        y[b] += r.results[core]["yT"].T
    y += b_proj[None, None, :]
    return y
